# revision 20
# baseline (speedup 1.0000x reference)
"""Trainium2 Bass kernel for CDAttnBlock.

Reference computation (per batch element b, all in fp32):
    q,k,v   = split(x  @ Wqkv)   heads=12, d=64
    q2,k2,v2= split(x2 @ Wqkv)
    o1 = attn(q, k,  v);  o2 = attn(q2, k2, v2);  o3 = attn(q, k2, v2)
    y_i = merge(o_i) @ Wout + bout

Sharding: pure data-parallel over batch (B=8) across 8 NeuronCores.
Each core runs the identical program on its own batch element; no
collectives.

Layout strategy (per core):
  - x.T in SBUF [768, 1024] built via PE transposes (contraction for all
    projections is over hidden, which must sit on partitions).
  - q.T, k.T stored per head-pair as [128, 1024] (d on partitions), v
    stored natural per s-tile as [128, 12*65] with a ones column per
    head appended (65th col) so the attention a@v matmul also produces
    the softmax denominator row.
  - scores computed transposed: sT[s_k, s_q] = kT.T @ qT, softmax as
    exp (no max subtraction: scores are N(0,~1), fp32 exp is safe),
    denominator via the ones column, normalization by a K=1 broadcast
    matmul + DVE multiply.
  - o accumulated transposed [768, 1024]; output projection uses oT as
    the stationary operand so y comes out natural [1024, 768].
  - Matmul operands are fp16 (1 cycle/row on the PE -- fp32 is 4x
    slower and fp32r 3x; fp16's 10-bit mantissa keeps rel err ~7e-4
    end-to-end). PSUM accumulation stays fp32. DMA-loaded weights are
    staged fp32 and cast on ScalarE; activations cast on DVE/ACT.
"""

import numpy as np

import concourse.bass as bass
import concourse.tile as tile
from concourse import bacc, mybir
from concourse.bass_utils import run_bass_kernel_spmd
from concourse.masks import make_identity

F32 = mybir.dt.float32
F16 = mybir.dt.float16
AF = mybir.ActivationFunctionType

HIDDEN = 768
HEADS = 12
D = 64
S = 1024
B = 8
SCALE = D ** -0.5
NPAIR = HEADS // 2          # 6 head pairs
KT = HIDDEN // 128          # 6 k-tiles over hidden
ST = S // 128               # 8 s-tiles
SQB = S // 512              # 2 s_q blocks of 512
VW = D + 1                  # 65: v columns + ones column


def _build_xt(nc, tc, x_ap, xT, ident):
    """DMA x natural and PE-transpose into f32r xT tiles [128, 1024] x 6."""
    xnat = tc.alloc_tile_pool(name="xnat", bufs=3, side="right")
    tpp = tc.alloc_tile_pool(name="tpp", bufs=2, space="PSUM")
    for st in range(ST):
        xn = xnat.tile([128, HIDDEN], F32, name="xn", tag="xn")
        nc.sync.dma_start(xn[:], x_ap[st * 128:(st + 1) * 128, :])
        for ht in range(KT):
            tp = tpp.tile([128, 128], F32, name="tp", tag="tp")
            nc.tensor.transpose(tp[:], xn[:, ht * 128:(ht + 1) * 128],
                                ident[:])
            nc.vector.tensor_copy(xT[ht][:, st * 128:(st + 1) * 128], tp[:])
    tpp.release()
    xnat.release()


def _qkv(nc, tc, w_dram, xT, qT, kT, v_st, onescol):
    """Project xT -> qT/kT per head-pair and v natural per s-tile."""
    # --- v for all heads: lhsT = xT tile, rhs = Wv slice [128, 768] ---
    wvp = tc.alloc_tile_pool(name="wvp", bufs=1, side="right")
    wvstage = tc.alloc_tile_pool(name="wvstage", bufs=2, side="right")
    wv = []
    for kt in range(KT):
        f = wvstage.tile([128, HIDDEN], F32, name="wvf", tag="wvf")
        nc.sync.dma_start(
            f[:], w_dram[kt * 128:(kt + 1) * 128, 2 * HIDDEN:3 * HIDDEN])
        t = wvp.tile([128, HIDDEN], F16, name=f"wv{kt}", tag=f"wv{kt}")
        nc.vector.tensor_copy(t[:], f[:])
        wv.append(t)
    vps = tc.alloc_tile_pool(name="vps", bufs=2, space="PSUM")
    for st in range(ST):
        vp = vps.tile([128, HIDDEN], F32, name="vp", tag="vp")
        for kt in range(KT):
            xts = xT[kt][:, st * 128:(st + 1) * 128]
            nc.tensor.matmul(vp[:, 0:512], xts, wv[kt][:, 0:512],
                             start=(kt == 0), stop=(kt == KT - 1))
            nc.tensor.matmul(vp[:, 512:768], xts, wv[kt][:, 512:768],
                             start=(kt == 0), stop=(kt == KT - 1))
        # scatter [128, 12, 64] -> v_st[:, h, 0:64]; ones col 65th
        vs = v_st[st]
        nc.vector.tensor_copy(
            vs.rearrange("p (h w) -> p h w", w=VW)[:, :, 0:D],
            vp.rearrange("p (h w) -> p h w", w=D))
        nc.vector.tensor_copy(
            vs.rearrange("p (h w) -> p h w", w=VW)[:, :, D:VW],
            onescol[:, None, :].broadcast_to([128, HEADS, 1]))
    vps.release()
    wvstage.release()
    wvp.release()

    # --- qT / kT per head pair: lhsT = Wq/Wk col slice, rhs = xT ---
    wsl = tc.alloc_tile_pool(name="wsl", bufs=2, side="right")
    wslstage = tc.alloc_tile_pool(name="wslstage", bufs=3, side="right")
    qkps = tc.alloc_tile_pool(name="qkps", bufs=2, space="PSUM")
    for p in range(NPAIR):
        for which, base, dst in ((0, 0, qT), (1, HIDDEN, kT)):
            ws = []
            for kt in range(KT):
                f = wslstage.tile([128, 128], F32, name="wslf", tag="wslf")
                nc.sync.dma_start(
                    f[:], w_dram[kt * 128:(kt + 1) * 128,
                                 base + p * 128:base + (p + 1) * 128])
                t = wsl.tile([128, 128], F16, name=f"wsl{which}{kt}",
                             tag=f"wsl{which}{kt}")
                nc.vector.tensor_copy(t[:], f[:])
                ws.append(t)
            pp = qkps.tile([128, S], F32, name="qkp", tag="qkp")
            for kt in range(KT):
                for nb in range(2):
                    nc.tensor.matmul(
                        pp[:, nb * 512:(nb + 1) * 512], ws[kt][:],
                        xT[kt][:, nb * 512:(nb + 1) * 512],
                        start=(kt == 0), stop=(kt == KT - 1))
            nc.vector.tensor_copy(dst[p][:], pp[:])
    qkps.release()
    wslstage.release()
    wsl.release()


def _attn(nc, tc, qT, kT, v_st, oT, e12, zbias):
    """oT[pair][64*hh:, :] = attention(q_h, k_h, v_h).T for both heads.

    Head pairs interleave inside the kt loop so the two heads' score
    matmuls sit on row-groups 0-63 / 64-127 and run concurrently on the
    PE. Unnormalized o and the denominator rows are staged to SBUF as
    each head finishes (freeing PSUM); all 12 reciprocals for the attn
    run as ONE DVE op at the end (reciprocal cost scales with free size
    only), then per-head one-hot selector matmuls against the [12, S]
    recip block broadcast each denominator row to 64 partitions.
    """
    sps = tc.alloc_tile_pool(name="sps", bufs=2, space="PSUM")
    ovps = tc.alloc_tile_pool(name="ovps", bufs=1, space="PSUM")
    exps = tc.alloc_tile_pool(name="exps", bufs=4, side="right")
    ovstage = tc.alloc_tile_pool(name="ovstage", bufs=4, side="right")
    smalls = tc.alloc_tile_pool(name="smalls", bufs=2, side="right")

    for g in range(NPAIR // 2):       # groups of 2 pairs = 4 heads
        den4 = smalls.tile([128, S], F32, name="den4", tag="den4")
        nc.vector.memset(den4[:], 1.0)
        ovs4 = []
        for p in (2 * g, 2 * g + 1):
            ovp = []
            for hh in range(2):
                ovp.append(ovps.tile([VW, S], F32, name=f"ov{hh}",
                                     tag=f"ov{hh}"))
            for kt in range(ST):
                exs = []
                for hh in range(2):
                    hp = slice(hh * D, (hh + 1) * D)
                    sp = sps.tile([128, S], F32, name=f"sp{hh}",
                                  tag=f"sp{hh}", bufs=1)
                    kts = kT[p][hp, kt * 128:(kt + 1) * 128]
                    for nb in range(2):
                        nc.tensor.matmul(
                            sp[:, nb * 512:(nb + 1) * 512], kts,
                            qT[p][hp, nb * 512:(nb + 1) * 512],
                            start=True, stop=True)
                    ex = exps.tile([128, S], F16, name="ex", tag="ex")
                    nc.scalar.activation(ex[:], sp[:], AF.Exp,
                                         bias=zbias[:], scale=SCALE)
                    exs.append(ex)
                for hh in range(2):
                    h = 2 * p + hh
                    vs = v_st[kt].rearrange("q (h w) -> q h w",
                                            w=VW)[:, h, :]
                    for nb in range(2):
                        nc.tensor.matmul(
                            ovp[hh][:, nb * 512:(nb + 1) * 512], vs,
                            exs[hh][:, nb * 512:(nb + 1) * 512],
                            start=(kt == 0), stop=(kt == ST - 1))
            for hh in range(2):
                gi = 2 * (p - 2 * g) + hh
                nc.vector.tensor_copy(den4[32 * gi:32 * gi + 1, :],
                                      ovp[hh][D:VW, :])
                ovs = ovstage.tile([D, S], F32, name="ovs", tag="ovs")
                nc.vector.tensor_copy(ovs[:], ovp[hh][0:D, :])
                ovs4.append(ovs)
        recf = smalls.tile([128, S], F32, name="recf", tag="recf")
        nc.vector.reciprocal(recf[:], den4[:])
        rec16 = smalls.tile([128, S], F16, name="rec16", tag="rec16")
        nc.vector.tensor_copy(rec16[:], recf[:])
        for p in (2 * g, 2 * g + 1):
            for hh in range(2):
                gi = 2 * (p - 2 * g) + hh
                hp = slice(hh * D, (hh + 1) * D)
                bc = ovps.tile([D, S], F32, name="bc", tag=f"ov{hh}")
                for nb in range(2):
                    nc.tensor.matmul(bc[:, nb * 512:(nb + 1) * 512],
                                     e12[gi],
                                     rec16[:, nb * 512:(nb + 1) * 512],
                                     start=True, stop=True)
                nc.vector.tensor_mul(oT[p][hp, :], ovs4[gi][:], bc[:])
    smalls.release()
    ovstage.release()
    exps.release()
    ovps.release()
    sps.release()


def _proj(nc, tc, oT, wout, bias_sb, y_dram):
    """y = oT.T @ Wout + bias, natural layout, DMA to DRAM."""
    yps = tc.alloc_tile_pool(name="yps", bufs=2, space="PSUM")
    ysb = tc.alloc_tile_pool(name="ysb", bufs=2, side="right")
    for st in range(ST):
        yp = yps.tile([128, HIDDEN], F32, name="yp", tag="yp")
        for ct in range(KT):
            ots = oT[ct][:, st * 128:(st + 1) * 128]
            nc.tensor.matmul(yp[:, 0:512], ots, wout[ct][:, 0:512],
                             start=(ct == 0), stop=(ct == KT - 1))
            nc.tensor.matmul(yp[:, 512:768], ots, wout[ct][:, 512:768],
                             start=(ct == 0), stop=(ct == KT - 1))
        yt = ysb.tile([128, HIDDEN], F32, name="yt", tag="yt")
        nc.vector.tensor_add(yt[:], yp[:], bias_sb[:])
        nc.sync.dma_start(y_dram[st * 128:(st + 1) * 128, :], yt[:])
    ysb.release()
    yps.release()


def build_kernel(ctx, tc, x, x2, wq, wo, bo, y1, y2, y3):
    nc = tc.nc

    const = ctx.enter_context(tc.tile_pool(name="const", bufs=1))
    ident = const.tile([128, 128], F32, name="ident")
    make_identity(nc, ident)
    zbias = const.tile([128, 1], F32, name="zbias")
    nc.vector.memset(zbias[:], 0.0)
    onescol = const.tile([128, 1], F32, name="onescol")
    nc.vector.memset(onescol[:], 1.0)
    # e12[gi]: [128, 64] fp16 selector with ones in row 32*gi — used as
    # matmul lhsT to broadcast row 32*gi of the recip block to 64 rows.
    e12 = []
    for gi in range(4):
        f = const.tile([128, D], F32, name=f"e4f{gi}")
        nc.vector.memset(f[:], 0.0)
        nc.vector.tensor_copy(
            f[32 * gi:32 * gi + 1, :],
            onescol[0:1, 0:1].broadcast_to((1, D)))
        t = const.tile([128, D], F16, name=f"e4{gi}")
        nc.vector.tensor_copy(t[:], f[:])
        e12.append(t)
    bias_sb = const.tile([128, HIDDEN], F32, name="bias_sb")
    bo_bcast = bass.AP(tensor=bo.tensor, offset=bo.offset,
                       ap=[[0, 128]] + list(bo.ap))
    nc.sync.dma_start(bias_sb[:], bo_bcast)

    woutp = ctx.enter_context(tc.tile_pool(name="woutp", bufs=1))
    wout = []
    for ct in range(KT):
        f = woutp.tile([128, HIDDEN], F32, name=f"woutf{ct}",
                       tag="woutf", bufs=2)
        nc.sync.dma_start(f[:], wo[ct * 128:(ct + 1) * 128, :])
        t = woutp.tile([128, HIDDEN], F16, name=f"wout{ct}", tag=f"wout{ct}")
        nc.vector.tensor_copy(t[:], f[:])
        wout.append(t)

    def persist(pool, shape, base, n, dtype=F16):
        return [pool.tile(shape, dtype, name=f"{base}{i}", tag=f"{base}{i}")
                for i in range(n)]

    # ---- persistent q for x (lives until o3) ----
    qxp = ctx.enter_context(tc.tile_pool(name="qxp", bufs=1))
    qT_x = persist(qxp, [128, S], "qTx", NPAIR)

    # ================= phase A: qkv for x =================
    kvxp = tc.alloc_tile_pool(name="kvxp", bufs=1)
    kT_x = persist(kvxp, [128, S], "kTx", NPAIR)
    v_x = persist(kvxp, [128, HEADS * VW], "vx", ST)

    xtp = tc.alloc_tile_pool(name="xtp", bufs=1)
    xT = persist(xtp, [128, S], "xT", KT)
    _build_xt(nc, tc, x, xT, ident)
    _qkv(nc, tc, wq, xT, qT_x, kT_x, v_x, onescol)
    xtp.release()

    # ================= phase B: o1 = attn(q, k, v); y1 =================
    o1p = tc.alloc_tile_pool(name="o1p", bufs=1)
    oT1 = persist(o1p, [128, S], "oT1", NPAIR)
    _attn(nc, tc, qT_x, kT_x, v_x, oT1, e12, zbias)
    _proj(nc, tc, oT1, wout, bias_sb, y1)
    o1p.release()
    kvxp.release()

    # ================= phase C: qkv for x2 =================
    kvx2p = tc.alloc_tile_pool(name="kvx2p", bufs=1)
    qT_x2 = persist(kvx2p, [128, S], "qTx2", NPAIR)
    kT_x2 = persist(kvx2p, [128, S], "kTx2", NPAIR)
    v_x2 = persist(kvx2p, [128, HEADS * VW], "vx2", ST)

    x2tp = tc.alloc_tile_pool(name="x2tp", bufs=1)
    x2T = persist(x2tp, [128, S], "x2T", KT)
    _build_xt(nc, tc, x2, x2T, ident)
    _qkv(nc, tc, wq, x2T, qT_x2, kT_x2, v_x2, onescol)
    x2tp.release()

    # ================= phase D: o2, y2, o3, y3 =================
    o2p = tc.alloc_tile_pool(name="o2p", bufs=1)
    oT2 = persist(o2p, [128, S], "oT2", NPAIR)
    _attn(nc, tc, qT_x2, kT_x2, v_x2, oT2, e12, zbias)
    _proj(nc, tc, oT2, wout, bias_sb, y2)
    o2p.release()

    o3p = tc.alloc_tile_pool(name="o3p", bufs=1)
    oT3 = persist(o3p, [128, S], "oT3", NPAIR)
    _attn(nc, tc, qT_x, kT_x2, v_x2, oT3, e12, zbias)
    _proj(nc, tc, oT3, wout, bias_sb, y3)
    o3p.release()
    kvx2p.release()


def build_bass():
    from contextlib import ExitStack
    nc = bacc.Bacc("TRN2", target_bir_lowering=False, debug=False,
                   num_devices=B)
    x = nc.dram_tensor("x", [S, HIDDEN], F32, kind="ExternalInput").ap()
    x2 = nc.dram_tensor("x2", [S, HIDDEN], F32, kind="ExternalInput").ap()
    wq = nc.dram_tensor("Wqkv", [HIDDEN, 3 * HIDDEN], F32,
                        kind="ExternalInput").ap()
    wo = nc.dram_tensor("Wout", [HIDDEN, HIDDEN], F32,
                        kind="ExternalInput").ap()
    bo = nc.dram_tensor("bout", [HIDDEN], F32, kind="ExternalInput").ap()
    y1 = nc.dram_tensor("y1", [S, HIDDEN], F32, kind="ExternalOutput").ap()
    y2 = nc.dram_tensor("y2", [S, HIDDEN], F32, kind="ExternalOutput").ap()
    y3 = nc.dram_tensor("y3", [S, HIDDEN], F32, kind="ExternalOutput").ap()
    with tile.TileContext(nc) as tc:
        with ExitStack() as ctx:
            build_kernel(ctx, tc, x, x2, wq, wo, bo, y1, y2, y3)
    nc.compile()
    return nc


def kernel(x, x2, Wqkv, Wout, bout):
    nc = build_bass()
    in_maps = [
        {"x": np.ascontiguousarray(x[b]), "x2": np.ascontiguousarray(x2[b]),
         "Wqkv": Wqkv, "Wout": Wout, "bout": bout}
        for b in range(B)
    ]
    res = run_bass_kernel_spmd(nc, in_maps, list(range(B)))
    y1 = np.stack([res.results[b]["y1"] for b in range(B)])
    y2 = np.stack([res.results[b]["y2"] for b in range(B)])
    y3 = np.stack([res.results[b]["y3"] for b in range(B)])
    return (y1, y2, y3)


# revision 33
# speedup vs baseline: 28.8411x; 28.8411x over previous
"""Trainium2 Bass kernel for CDAttnBlock.

Reference computation (per batch element b, all in fp32):
    q,k,v   = split(x  @ Wqkv)   heads=12, d=64
    q2,k2,v2= split(x2 @ Wqkv)
    o1 = attn(q, k,  v);  o2 = attn(q2, k2, v2);  o3 = attn(q, k2, v2)
    y_i = merge(o_i) @ Wout + bout

Sharding: pure data-parallel over batch (B=8) across 8 NeuronCores;
each core runs the identical program on its own batch element, no
collectives.

Per-core design:
  - All matmul operands are fp16 (full 1 cycle/row PE rate; fp32 is 4x
    slower, fp32r 3x; fp16 keeps end-to-end rel err ~7e-4). PSUM
    accumulation is fp32.
  - x.T built via PE transposes into one [128, 6*1024] fp16 tile
    (hidden on partitions); q.T/k.T per head-pair [128, 1024]; v per
    s-tile [128, 12*65] with a ones column per head so the a@v matmul
    also emits the softmax denominator row.
  - Scores are computed transposed (sT = kT.T @ qT) so softmax needs no
    on-chip transposes; exp runs on ScalarE straight out of PSUM with
    the 1/sqrt(d) scale folded in; no max-subtraction (scores ~N(0,1)).
  - Denominators from 4 heads are collected at partitions {0,32,64,96},
    inverted in ONE DVE reciprocal (its cost scales only with the free
    size), broadcast to 64 partitions on the idle GpSimd engine
    (partition_broadcast), and applied with a fp16 DVE multiply.
  - o accumulates transposed so the output projection (oT as stationary
    operand) yields y in natural [1024, 768] layout.
  - The attention phases are ScalarE(exp)-bound; leaving the PE idle
    there makes the HAM clock-gate drop it to 1.2 GHz. So all other PE
    work (x2 transposes + its qkv projections, then the output
    projections) is chopped into small thunks and interleaved into the
    attention instruction stream to keep the PE continuously busy.
"""

import numpy as np

import concourse.bass as bass
import concourse.tile as tile
from concourse import bacc, mybir
from concourse.bass_utils import run_bass_kernel_spmd
from concourse.masks import make_identity

F32 = mybir.dt.float32
F16 = mybir.dt.float16
AF = mybir.ActivationFunctionType

HIDDEN = 768
HEADS = 12
D = 64
S = 1024
B = 8
SCALE = D ** -0.5
NPAIR = HEADS // 2          # 6 head pairs
KT = HIDDEN // 128          # 6 k-tiles over hidden
ST = S // 128               # 8 s-tiles
VW = D + 1                  # 65: v columns + ones column


class Ctx:
    """Shared handles for the kernel builder."""


def _emit_xt(c, x_ap, xT, xnat, psum_pool, psum_tag, thunks=None):
    """Build xT [128, KT*S] fp16 from x [S, H]: DMA natural tiles, PE
    transpose 6 blocks per s-tile into one psum tile, one DVE evac."""
    nc = c.nc
    out3 = xT.rearrange("p (h s) -> p h s", s=S)
    xns = {}

    def dma(st):
        def f():
            xn = xnat.tile([128, HIDDEN], F32, name="xn", tag="xn")
            xns[st] = xn
            nc.sync.dma_start(xn[:], x_ap[st * 128:(st + 1) * 128, :])
        return f

    def tp(st, half):
        def f():
            tag = psum_tag[half % len(psum_tag)] if isinstance(
                psum_tag, (list, tuple)) else psum_tag
            pt = psum_pool.tile([128, 3 * 128], F32, name="tpp", tag=tag)
            for i in range(3):
                ht = 3 * half + i
                nc.tensor.transpose(
                    pt[:, i * 128:(i + 1) * 128],
                    xns[st][:, ht * 128:(ht + 1) * 128], c.ident[:])
            nc.vector.tensor_copy(
                out3[:, 3 * half:3 * half + 3, st * 128:(st + 1) * 128],
                pt.rearrange("p (h s) -> p h s", s=128))
        return f

    for st in range(ST):
        for f in (dma(st), tp(st, 0), tp(st, 1)):
            if thunks is None:
                f()
            else:
                thunks.append(f)


def _emit_qkv(c, xT, qT, kT, v_st, psum_pool, psum_tag, thunks=None,
              parts=("v", "q", "k")):
    """xT [128, KT*S] fp16 -> qT/kT per pair [128, S] fp16 and v per
    s-tile [128, 12*65] fp16 (with ones column). `parts` selects which
    of v/q/k to emit."""
    nc = c.nc

    def xts(kt, a, b):
        return xT[:, kt * S + a:kt * S + b]

    # ---- v: out [s-tile, 768] accumulated over kt ----
    def v_half(st, half):
        def f():
            tag = psum_tag[half % len(psum_tag)] if isinstance(
                psum_tag, (list, tuple)) else psum_tag
            lo, hi = (0, 512) if half == 0 else (512, 768)
            vp = psum_pool.tile([128, hi - lo], F32, name="vp", tag=tag)
            for kt in range(KT):
                nc.tensor.matmul(
                    vp[:], xts(kt, st * 128, (st + 1) * 128),
                    c.wq16[kt][:, 2 * HIDDEN + lo:2 * HIDDEN + hi],
                    start=(kt == 0), stop=(kt == KT - 1))
            vs3 = v_st[st].rearrange("p (h w) -> p h w", w=VW)
            ha, hb = (0, 8) if half == 0 else (8, 12)
            nc.vector.tensor_copy(
                vs3[:, ha:hb, 0:D],
                vp.rearrange("p (h w) -> p h w", w=D))
            if half == 1:
                nc.vector.tensor_copy(
                    vs3[:, :, D:VW],
                    c.onescol[:, None, :].broadcast_to([128, HEADS, 1]))
        return f

    if "v" in parts:
        for st in range(ST):
            for half in range(2):
                f = v_half(st, half)
                if thunks is None:
                    f()
                else:
                    thunks.append(f)

    # ---- qT / kT per pair: lhsT = Wq/Wk col slice, rhs = xT ----
    def qk_half(p, base, dst, half, hold):
        def f():
            tag = psum_tag[half % len(psum_tag)] if isinstance(
                psum_tag, (list, tuple)) else psum_tag
            pp = psum_pool.tile([128, 512], F32, name="qkp", tag=tag)
            lo = half * 512
            for kt in range(KT):
                nc.tensor.matmul(
                    pp[:],
                    c.wq16[kt][:, base + p * 128:base + (p + 1) * 128],
                    xts(kt, lo, lo + 512),
                    start=(kt == 0), stop=(kt == KT - 1))
            nc.vector.tensor_copy(dst[p][:, lo:lo + 512], pp[:])
        return f

    sel = [(0, qT)] * ("q" in parts) + [(HIDDEN, kT)] * ("k" in parts)
    for p in range(NPAIR):
        for base, dst in sel:
            hold = {}
            for half in range(2):
                f = qk_half(p, base, dst, half, hold)
                if thunks is None:
                    f()
                else:
                    thunks.append(f)


def _emit_proj(c, oT, y_dram, psum_pool, psum_tag, thunks=None):
    """y = oT.T @ Wout + bias -> DRAM, natural [S, H] layout."""
    nc = c.nc

    def half(st, h, hold):
        def f():
            tag = psum_tag[h % len(psum_tag)] if isinstance(
                psum_tag, (list, tuple)) else psum_tag
            lo, hi = (0, 512) if h == 0 else (512, 768)
            yp = psum_pool.tile([128, hi - lo], F32, name="yp", tag=tag)
            for ct in range(KT):
                nc.tensor.matmul(
                    yp[:], oT[ct][:, st * 128:(st + 1) * 128],
                    c.wout16[ct][:, lo:hi],
                    start=(ct == 0), stop=(ct == KT - 1))
            if h == 0:
                hold["yt"] = c.ysb.tile([128, HIDDEN], F32, name="yt",
                                        tag="yt")
            yt = hold["yt"]
            nc.vector.tensor_add(yt[:, lo:hi], yp[:],
                                 c.bias_sb[:, lo:hi])
            if h == 1:
                nc.sync.dma_start(y_dram[st * 128:(st + 1) * 128, :], yt[:])
        return f

    for st in range(ST):
        hold = {}
        for h in range(2):
            f = half(st, h, hold)
            if thunks is None:
                f()
            else:
                thunks.append(f)


def _attn(c, tc, qT, kT, v_st, oT, thunks, npump=None):
    """One attention (12 heads, one at a time). `thunks` (aux PE work +
    deferred normalize chains) are pumped into the exp-wait gaps at an
    even pace so the PE stays busy (and the HAM clock-gate warm) for the
    whole phase."""
    nc = c.nc
    work = list(thunks)          # PE-heavy aux thunks
    dveq = []                    # deferred normalize (DVE/gpsimd only)
    state = {"i": 0, "credit": 0.0, "j": 0}
    # pump sites: one per kt per head (96) plus one per head tail (12)
    sites = HEADS * ST + HEADS
    rate = None

    def pump(k=1.0):
        nonlocal rate
        if rate is None:
            rate = max(1.0, len(work) / sites)
        state["credit"] += k * rate
        while state["credit"] >= 1.0 and state["i"] < len(work):
            work[state["i"]]()
            state["i"] += 1
            state["credit"] -= 1.0

    def pump_dve(n=1):
        for _ in range(n):
            if state["j"] < len(dveq):
                dveq[state["j"]]()
                state["j"] += 1

    def drain():
        while state["i"] < len(work):
            work[state["i"]]()
            state["i"] += 1
        while state["j"] < len(dveq):
            dveq[state["j"]]()
            state["j"] += 1

    sps = tc.alloc_tile_pool(name="sps", bufs=2, space="PSUM")
    ovps = tc.alloc_tile_pool(name="ovps", bufs=1, space="PSUM")
    exps = tc.alloc_tile_pool(name="exps", bufs=4, side="right")
    ovstage = tc.alloc_tile_pool(name="ovstage", bufs=5, side="right")
    smalls = tc.alloc_tile_pool(name="smalls", bufs=2, side="right")

    def normalize_thunks(den4, ovs4, g):
        """Deferred DVE/gpsimd normalize chain for one 4-head group."""
        hold = {}

        def t_recip():
            recf = smalls.tile([128, S], F32, name="recf", tag="recf")
            nc.vector.reciprocal(recf[:], den4[:])
            hold["recf"] = recf

        def t_head(gi):
            def f():
                h = 4 * g + gi
                p, hh = h // 2, h % 2
                hp = slice(hh * D, (hh + 1) * D)
                rrow = smalls.tile([1, S], F16, name="rrow", tag="rrow",
                                   bufs=2)
                nc.vector.tensor_copy(
                    rrow[:], hold["recf"][32 * gi:32 * gi + 1, :])
                bcs = smalls.tile([D, S], F16, name="bcs", tag="bcs",
                                  bufs=2)
                nc.gpsimd.partition_broadcast(bcs[:], rrow[:])
                nc.vector.tensor_mul(oT[p][hp, :], ovs4[gi][:], bcs[:])
            return f

        return [t_recip] + [t_head(gi) for gi in range(4)]

    for g in range(HEADS // 4):       # 3 groups of 4 heads
        den4 = smalls.tile([128, S], F32, name="den4", tag="den4")
        nc.vector.memset(den4[:], 1.0)
        ovs4 = []
        for gi in range(4):
            h = 4 * g + gi
            p, hh = h // 2, h % 2
            hp = slice(hh * D, (hh + 1) * D)
            ov = ovps.tile([VW, S], F32, name="ov", tag="ov")
            for kt in range(ST):
                sp = sps.tile([128, S], F32, name="sp", tag="sp")
                kts = kT[p][hp, kt * 128:(kt + 1) * 128]
                for nb in range(2):
                    nc.tensor.matmul(
                        sp[:, nb * 512:(nb + 1) * 512], kts,
                        qT[p][hp, nb * 512:(nb + 1) * 512],
                        start=True, stop=True)
                ex = exps.tile([128, S], F16, name="ex", tag="ex")
                nc.scalar.activation(ex[:], sp[:], AF.Exp,
                                     bias=c.zbias[:], scale=SCALE)
                vs = v_st[kt].rearrange("q (h w) -> q h w", w=VW)[:, h, :]
                for nb in range(2):
                    nc.tensor.matmul(
                        ov[:, nb * 512:(nb + 1) * 512], vs,
                        ex[:, nb * 512:(nb + 1) * 512],
                        start=(kt == 0), stop=(kt == ST - 1))
                pump(1.0)
                if kt < 3:
                    # normalize items only early in the head, keeping the
                    # DVE queue clear for the staging copies that gate the
                    # next head's PSUM slot
                    pump_dve(1)
            nc.vector.tensor_copy(den4[32 * gi:32 * gi + 1, :],
                                  ov[D:VW, :])
            ovs = ovstage.tile([D, S], F16, name="ovs", tag="ovs")
            nc.vector.tensor_copy(ovs[:], ov[0:D, :])
            ovs4.append(ovs)
            pump(1.0)
        dveq.extend(normalize_thunks(den4, ovs4, g))
    drain()
    smalls.release()
    ovstage.release()
    exps.release()
    ovps.release()
    sps.release()


def build_kernel(ctx, tc, x, x2, wq, wo, bo, y1, y2, y3):
    nc = tc.nc
    c = Ctx()
    c.nc = nc

    # ---------------- constants + weights (fp16 resident) -------------
    const = ctx.enter_context(tc.tile_pool(name="const", bufs=1))
    c.ident = const.tile([128, 128], F32, name="ident")
    make_identity(nc, c.ident)
    c.zbias = const.tile([128, 1], F32, name="zbias")
    nc.vector.memset(c.zbias[:], 0.0)
    c.onescol = const.tile([128, 1], F32, name="onescol")
    nc.vector.memset(c.onescol[:], 1.0)
    c.bias_sb = const.tile([128, HIDDEN], F32, name="bias_sb")
    bo_bcast = bass.AP(tensor=bo.tensor, offset=bo.offset,
                       ap=[[0, 128]] + list(bo.ap))
    nc.sync.dma_start(c.bias_sb[:], bo_bcast)

    wstage = tc.alloc_tile_pool(name="wstage", bufs=2, side="right")
    woutp = ctx.enter_context(tc.tile_pool(name="woutp", bufs=1))
    c.wout16 = []
    for ct in range(KT):
        f = wstage.tile([128, HIDDEN], F32, name="wof", tag="wof")
        nc.sync.dma_start(f[:], wo[ct * 128:(ct + 1) * 128, :])
        t = woutp.tile([128, HIDDEN], F16, name=f"wout{ct}", tag=f"wout{ct}")
        nc.vector.tensor_copy(t[:], f[:])
        c.wout16.append(t)

    def persist(pool, shape, base, n, dtype=F16):
        return [pool.tile(shape, dtype, name=f"{base}{i}", tag=f"{base}{i}")
                for i in range(n)]

    qxp = ctx.enter_context(tc.tile_pool(name="qxp", bufs=1))
    qT_x = persist(qxp, [128, S], "qTx", NPAIR)
    # x2's qkv allocated before kvxp so kvxp can release first (LIFO)
    kvx2p = ctx.enter_context(tc.tile_pool(name="kvx2p", bufs=1))
    qT_x2 = persist(kvx2p, [128, S], "qTx2", NPAIR)
    kT_x2 = persist(kvx2p, [128, S], "kTx2", NPAIR)
    v_x2 = persist(kvx2p, [128, HEADS * VW], "vx2", ST)
    # oT slots: tag "oTa" holds oT1 then oT3; "oTb" holds oT2
    otp = ctx.enter_context(tc.tile_pool(name="otp", bufs=1))
    oT1 = persist(otp, [128, S], "oTa", NPAIR)
    kvxp = tc.alloc_tile_pool(name="kvxp", bufs=1)
    kT_x = persist(kvxp, [128, S], "kTx", NPAIR)
    v_x = persist(kvxp, [128, HEADS * VW], "vx", ST)
    wqp = tc.alloc_tile_pool(name="wqp", bufs=1)
    c.wq16 = []
    for kt in range(KT):
        f = wstage.tile([128, 3 * HIDDEN], F32, name="wqf", tag="wqf")
        nc.sync.dma_start(f[:], wq[kt * 128:(kt + 1) * 128, :])
        t = wqp.tile([128, 3 * HIDDEN], F16, name=f"wq16{kt}",
                     tag=f"wq16{kt}")
        nc.vector.tensor_copy(t[:], f[:])
        c.wq16.append(t)
    wstage.release()
    c.ysb = ctx.enter_context(tc.tile_pool(name="ysb", bufs=2, side="right"))

    # ---------------- phase 1: xT + qkv(x), dense ---------------------
    xnat = tc.alloc_tile_pool(name="xnat", bufs=2, side="right")
    p1ps = tc.alloc_tile_pool(name="p1ps", bufs=2, space="PSUM")
    xtp = tc.alloc_tile_pool(name="xtp", bufs=1)
    xT = xtp.tile([128, KT * S], F16, name="xT")
    _emit_xt(c, x, xT, xnat, p1ps, "p1")
    _emit_qkv(c, xT, qT_x, kT_x, v_x, p1ps, "p1")
    xtp.release()
    p1ps.release()

    # ---- phase 2: attn(o1), aux = x2T + k2 + v2 ----------------------
    auxp = tc.alloc_tile_pool(name="auxp", bufs=1, space="PSUM")
    AUXT = ["auxA", "auxB"]
    x2tp = tc.alloc_tile_pool(name="x2tp", bufs=1)
    x2T = x2tp.tile([128, KT * S], F16, name="x2T")
    thunks2 = []
    _emit_xt(c, x2, x2T, xnat, auxp, AUXT, thunks=thunks2)
    _emit_qkv(c, x2T, qT_x2, kT_x2, v_x2, auxp, AUXT, thunks=thunks2,
              parts=("v", "k"))
    _attn(c, tc, qT_x, kT_x, v_x, oT1, thunks2)

    # ---- phase 3: attn(o3), aux = q2 + proj(y1) ----------------------
    oT3 = persist(otp, [128, S], "oTb", NPAIR)
    thunks3 = []
    _emit_qkv(c, x2T, qT_x2, kT_x2, v_x2, auxp, AUXT, thunks=thunks3,
              parts=("q",))
    _emit_proj(c, oT1, y1, auxp, AUXT, thunks=thunks3)
    _attn(c, tc, qT_x, kT_x2, v_x2, oT3, thunks3)
    x2tp.release()
    wqp.release()
    kvxp.release()

    # ---- phase 4: attn(o2), aux = proj(y3) ---------------------------
    oT2 = persist(otp, [128, S], "oTa", NPAIR)
    thunks4 = []
    _emit_proj(c, oT3, y3, auxp, AUXT, thunks=thunks4)
    _attn(c, tc, qT_x2, kT_x2, v_x2, oT2, thunks4)

    # ---- phase 5: proj(y2) -------------------------------------------
    _emit_proj(c, oT2, y2, auxp, AUXT)
    auxp.release()
    xnat.release()


def build_bass():
    from contextlib import ExitStack
    nc = bacc.Bacc("TRN2", target_bir_lowering=False, debug=False,
                   num_devices=B)
    x = nc.dram_tensor("x", [S, HIDDEN], F32, kind="ExternalInput").ap()
    x2 = nc.dram_tensor("x2", [S, HIDDEN], F32, kind="ExternalInput").ap()
    wq = nc.dram_tensor("Wqkv", [HIDDEN, 3 * HIDDEN], F32,
                        kind="ExternalInput").ap()
    wo = nc.dram_tensor("Wout", [HIDDEN, HIDDEN], F32,
                        kind="ExternalInput").ap()
    bo = nc.dram_tensor("bout", [HIDDEN], F32, kind="ExternalInput").ap()
    y1 = nc.dram_tensor("y1", [S, HIDDEN], F32, kind="ExternalOutput").ap()
    y2 = nc.dram_tensor("y2", [S, HIDDEN], F32, kind="ExternalOutput").ap()
    y3 = nc.dram_tensor("y3", [S, HIDDEN], F32, kind="ExternalOutput").ap()
    with tile.TileContext(nc) as tc:
        with ExitStack() as ctx:
            build_kernel(ctx, tc, x, x2, wq, wo, bo, y1, y2, y3)
    nc.compile()
    return nc


_NC_CACHE = []


def kernel(x, x2, Wqkv, Wout, bout):
    if not _NC_CACHE:
        _NC_CACHE.append(build_bass())
    nc = _NC_CACHE[0]
    in_maps = [
        {"x": np.ascontiguousarray(x[b]), "x2": np.ascontiguousarray(x2[b]),
         "Wqkv": Wqkv, "Wout": Wout, "bout": bout}
        for b in range(B)
    ]
    res = run_bass_kernel_spmd(nc, in_maps, list(range(B)))
    y1 = np.stack([res.results[b]["y1"] for b in range(B)])
    y2 = np.stack([res.results[b]["y2"] for b in range(B)])
    y3 = np.stack([res.results[b]["y3"] for b in range(B)])
    return (y1, y2, y3)
